# revision 13
# baseline (speedup 1.0000x reference)
"""DynamicCrossAttention Trainium2 kernel (per-core builder + host wrapper).

Sharding: 8 shards = (B=4 batches) x (N=4096 query rows split in 2).
Each core: 2048 query rows of one batch, full context of that batch.

Key structure (v2):
  - K projection eliminated: S = xn @ WQK @ cn^T with WQK = (g1*Wq)(g2*Wk)^T/sqrt(C)
    folded on host (plus exact rank-1 bias corrections when biases nonzero).
  - fp8 (e4m3) DoubleRow matmuls for Qk / VP / MLP / scores / attn-out.
  - Top-k: scores drained by ACT as Exp(s) in bf16 into the HIGH u16 halves of a
    u32 tile whose LOW halves hold a column-index iota. One DVE Max8 over the
    u32 view returns (value|index) pairs: no MaxIndex scan needed.
  - Weights: w_j = e^{v_j}-1 (thresholded); denominator = M + sum w.
    out = (sum_j w_j VP[idx_j] + sum_r VP_r)/den + x.
"""

import math
import sys

sys.path.insert(0, "/opt/trn_rl_repo")

import numpy as np
import ml_dtypes

import concourse.bass as bass
import concourse.tile as tile
import concourse.mybir as mybir
from concourse import bacc
from concourse.bass import IndirectOffsetOnAxis
from concourse.masks import make_identity

F32 = mybir.dt.float32
BF16 = mybir.dt.bfloat16
FP8 = mybir.dt.float8e4
U32 = mybir.dt.uint32
U16 = mybir.dt.uint16
AF = mybir.ActivationFunctionType
ALU = mybir.AluOpType
PM = mybir.MatmulPerfMode

P = 128
D = 512
H = 128  # threshold MLP hidden
NQ = 2048  # query rows per core
M = 4096  # context rows
K5 = 5
EPS = 1e-5
NCH = D // P  # 4 feature chunks
NTQ = NQ // P  # 16 query tiles
NTC = M // P  # 32 context row tiles

SQ = 64.0  # scale folded into WQK (scores psum = SQ * s)
SV = 4.0  # scale folded into WVP (vp rows = SV * VP)
ST = 16.0  # scale folded into Wt1


def build_core_program(tc, flags):
    add_bias_out, has_u = flags
    nc = tc.nc

    xs = nc.dram_tensor("xs", [NQ, D], F32, kind="ExternalInput").ap()
    ctx = nc.dram_tensor("ctx", [M, D], F32, kind="ExternalInput").ap()
    wqk_d = nc.dram_tensor("wqk", [D, D], FP8, kind="ExternalInput").ap()
    wvp_d = nc.dram_tensor("wvp", [D, D], FP8, kind="ExternalInput").ap()
    wvpb_d = nc.dram_tensor("wvpb", [D, D], BF16, kind="ExternalInput").ap()
    wt1_d = nc.dram_tensor("wt1", [D, H], FP8, kind="ExternalInput").ap()
    wt2_d = nc.dram_tensor("wt2", [H, 1], BF16, kind="ExternalInput").ap()
    bqk_d = nc.dram_tensor("bqk", [D, 1], F32, kind="ExternalInput").ap()
    bt1_d = nc.dram_tensor("bt1", [H, 1], F32, kind="ExternalInput").ap()
    bt2_d = nc.dram_tensor("bt2", [P, 1], F32, kind="ExternalInput").ap()
    bo_d = nc.dram_tensor("bo", [1, D], BF16, kind="ExternalInput").ap()
    uv_d = nc.dram_tensor("uv", [D, 1], BF16, kind="ExternalInput").ap()
    out = nc.dram_tensor("out", [NQ, D], F32, kind="ExternalOutput").ap()
    vpd = nc.dram_tensor("vpd", [M, D], FP8, kind="Internal").ap()

    from contextlib import ExitStack
    es = ExitStack()
    const = es.enter_context(tc.tile_pool(name="const", bufs=1))
    wpool = es.enter_context(tc.tile_pool(name="wpool", bufs=1))
    big = es.enter_context(tc.tile_pool(name="big", bufs=1))
    ldpool = es.enter_context(tc.tile_pool(name="ld", bufs=8))
    stats = es.enter_context(tc.tile_pool(name="stats", bufs=8))
    vpool = es.enter_context(tc.tile_pool(name="vpool", bufs=3))
    fpool = es.enter_context(tc.tile_pool(name="fpool", bufs=4))
    gpool = es.enter_context(tc.tile_pool(name="gpool", bufs=3))
    opool = es.enter_context(tc.tile_pool(name="op", bufs=3))
    psA = es.enter_context(tc.tile_pool(name="psA", bufs=2, space="PSUM"))
    psB = es.enter_context(tc.tile_pool(name="psB", bufs=2, space="PSUM"))
    psT = es.enter_context(tc.tile_pool(name="psT", bufs=2, space="PSUM"))

    # ---- constants ----
    ident8 = const.tile([P, P], FP8, name="ident8")
    make_identity(nc, ident8[:])
    identb = const.tile([P, P], BF16, name="identb")
    make_identity(nc, identb[:])
    ones_row = const.tile([1, P], BF16, name="ones_row")
    nc.vector.memset(ones_row[:], 1.0)
    eps_c = const.tile([P, 1], F32, name="eps_c")
    nc.vector.memset(eps_c[:], EPS)

    # ---- weights to SBUF ----
    wqk_sb = wpool.tile([P, NCH, D], FP8, name="wqk_sb")
    nc.scalar.dma_start(wqk_sb[:], wqk_d.rearrange("(c p) d -> p c d", p=P))
    wvp_sb = wpool.tile([P, NCH, D], FP8, name="wvp_sb")
    nc.scalar.dma_start(wvp_sb[:], wvp_d.rearrange("(c p) d -> p c d", p=P))
    wvpb_sb = wpool.tile([P, NCH, D], BF16, name="wvpb_sb")
    nc.scalar.dma_start(wvpb_sb[:], wvpb_d.rearrange("(c p) d -> p c d", p=P))
    wt1_sb = wpool.tile([P, NCH, H], FP8, name="wt1_sb")
    nc.scalar.dma_start(wt1_sb[:], wt1_d.rearrange("(c p) d -> p c d", p=P))
    wt2_sb = wpool.tile([H, 1], BF16, name="wt2_sb")
    nc.scalar.dma_start(wt2_sb[:], wt2_d)
    bqk_sb = wpool.tile([P, NCH], F32, name="bqk_sb")
    nc.scalar.dma_start(bqk_sb[:], bqk_d.rearrange("(c p) o -> p (c o)", p=P))
    bt1_sb = wpool.tile([H, 1], F32, name="bt1_sb")
    nc.scalar.dma_start(bt1_sb[:], bt1_d)
    bt2_sb = wpool.tile([P, 1], F32, name="bt2_sb")
    nc.scalar.dma_start(bt2_sb[:], bt2_d)
    bo_sb = wpool.tile([1, D], BF16, name="bo_sb")
    nc.scalar.dma_start(bo_sb[:], bo_d)
    uv_sb = wpool.tile([P, NCH], BF16, name="uv_sb")
    nc.scalar.dma_start(uv_sb[:], uv_d.rearrange("(c p) o -> p (c o)", p=P))

    # ---- persistent tensors ----
    xraw = big.tile([P, NTQ, D], F32, name="xraw")
    cnA = big.tile([P, NTC, D], BF16, name="cnA")  # normalized ctx (row-major)
    xnA = big.tile([P, NTQ, D], BF16, name="xnA")  # normalized x (row-major)
    ctxcT = big.tile([P, NCH, M], FP8, name="ctxcT")
    qkT = big.tile([P, NCH, NQ], FP8, name="qkT")
    hT = big.tile([P, NQ], BF16, name="hT")
    tcols = big.tile([P, NTQ], F32, name="tcols")
    u_sb = big.tile([P, NTQ], F32, name="u_sb")
    et = big.tile([P, NTQ], F32, name="et")
    eu = big.tile([P, NTQ], F32, name="eu")
    sparts = big.tile([P, NCH, NTC // 4], F32, name="sparts")
    sctx = big.tile([P, NCH], F32, name="sctx")
    sctx_bf = big.tile([P, NCH], BF16, name="sctx_bf")
    svp_row = big.tile([1, D], BF16, name="svp_row")
    senc0 = big.tile([P, M], U32, name="senc0")
    senc1 = big.tile([P, M], U32, name="senc1")

    # index iota (idx << 7) into the score-encode buffers; the Exp drain
    # overwrites byte 3 of each word with the fp8 value, bits 0..6 stay zero
    # so DVE Max8's internal mantissa rounding (step <= 128) is a no-op.
    for senc in (senc0, senc1):
        nc.gpsimd.iota(senc[:], pattern=[[128, M]], base=0,
                       channel_multiplier=0)

    if add_bias_out:
        psb = psB.tile([P, D], F32, name="psB_t")
        nc.tensor.matmul(psb[:], lhsT=ones_row[:], rhs=bo_sb[:], start=True,
                         stop=True)
        bob_sb = big.tile([P, D], BF16, name="bob_sb")
        nc.scalar.activation(bob_sb[:], psb[:], AF.Copy, bias=0.0)

    # ==================== Phase 1: layernorm stats+apply ====================
    def ln_group(raws, outs):
        mv4 = stats.tile([P, 4, 2], F32, name="mv4")
        for b in range(4):
            bn6 = stats.tile([P, 6], F32, name="bn6")
            nc.vector.bn_stats(bn6[:], raws[b])
            nc.vector.bn_aggr(mv4[:, b, :], bn6[:])
        sd4 = stats.tile([P, 4], F32, name="sd4")
        nc.scalar.activation(sd4[:], mv4[:, :, 1], AF.Sqrt, bias=eps_c[:])
        rstd4 = stats.tile([P, 4], F32, name="rstd4")
        nc.vector.reciprocal(rstd4[:], sd4[:])
        nmr4 = stats.tile([P, 4], F32, name="nmr4")
        nc.vector.tensor_mul(nmr4[:], mv4[:, :, 0], rstd4[:])
        nc.vector.tensor_scalar_mul(nmr4[:], nmr4[:], -1.0)
        for b in range(4):
            nc.vector.tensor_scalar(outs[b], raws[b], rstd4[:, b:b + 1],
                                    nmr4[:, b:b + 1], op0=ALU.mult, op1=ALU.add)

    for g in range(NTC // 4):
        raw_list = []
        for b in range(4):
            jt = g * 4 + b
            raw = ldpool.tile([P, D], F32, name="ldraw")
            nc.sync.dma_start(raw[:], ctx[jt * P:(jt + 1) * P, :])
            raw_list.append(raw[:])
        ln_group(raw_list, [cnA[:, g * 4 + b, :] for b in range(4)])

    for g in range(NTQ // 4):
        raw_list = []
        for b in range(4):
            it = g * 4 + b
            nc.sync.dma_start(xraw[:, it, :], xs[it * P:(it + 1) * P, :])
            raw_list.append(xraw[:, it, :])
        ln_group(raw_list, [xnA[:, g * 4 + b, :] for b in range(4)])

    # ==================== Phase 2: transposes + projections ====================
    # ctx transposes -> ctxcT (fp8), with column-sum accumulation for svp
    for g in range(NTC // 4):
        for c in range(NCH):
            pt = psT.tile([P, D], BF16, name="pt")
            for b in range(4):
                nc.tensor.transpose(pt[:, b * P:(b + 1) * P],
                                    cnA[:, g * 4 + b, c * P:(c + 1) * P],
                                    identb[:])
            nc.scalar.activation(ctxcT[:, c, g * D:(g + 1) * D], pt[:],
                                 AF.Copy, bias=0.0,
                                 accum_out=sparts[:, c, g:g + 1])

    # VP = cn @ WVP (scaled by SV), row-major to DRAM (fp8)
    for jp in range(NTC // 2):
        ps = psA.tile([P, 2 * D], F32, name="psA_t")
        for half in range(2):
            jt = jp * 2 + half
            for cp in range(NCH // 2):
                nc.tensor.matmul(ps[:, half * D:(half + 1) * D],
                                 lhsT=ctxcT[:, 2 * cp:2 * cp + 2,
                                            jt * P:(jt + 1) * P],
                                 rhs=wvp_sb[:, 2 * cp:2 * cp + 2, :],
                                 start=(cp == 0), stop=(cp == NCH // 2 - 1),
                                 perf_mode=PM.DoubleRow)
        vp_sb = vpool.tile([P, 2 * D], FP8, name="vp_sb")
        nc.scalar.activation(vp_sb[:], ps[:], AF.Copy, bias=0.0)
        nc.sync.dma_start(
            vpd[jp * 2 * P:(jp + 1) * 2 * P, :].rearrange(
                "(two p) d -> p two d", p=P),
            vp_sb[:])

    # svp_row = (column-sums of cn) @ WVP   (bf16 path for precision)
    for c in range(NCH):
        nc.vector.reduce_sum(sctx[:, c:c + 1], sparts[:, c, :],
                             axis=mybir.AxisListType.X)
    nc.vector.tensor_copy(sctx_bf[:], sctx[:])
    ps = psB.tile([P, D], F32, name="psB_t")
    for ci in range(NCH):
        nc.tensor.matmul(ps[:1, :], lhsT=sctx_bf[:, ci:ci + 1],
                         rhs=wvpb_sb[:, ci, :],
                         start=(ci == 0), stop=(ci == NCH - 1))
    nc.scalar.activation(svp_row[:], ps[:1, :], AF.Copy, bias=0.0)

    # x transposes -> xcT; Qk projection; threshold MLP
    xcT = big.tile([P, NCH, NQ], FP8, name="xcT")
    for g in range(NTQ // 4):
        for c in range(NCH):
            pt = psT.tile([P, D], BF16, name="pt")
            for b in range(4):
                nc.tensor.transpose(pt[:, b * P:(b + 1) * P],
                                    xnA[:, g * 4 + b, c * P:(c + 1) * P],
                                    identb[:])
            nc.scalar.activation(xcT[:, c, g * D:(g + 1) * D], pt[:],
                                 AF.Copy, bias=0.0)
        # Qk^T chunks for this 512-col group
        for c2 in range(NCH):
            ps = psB.tile([P, D], F32, name="psB_t")
            for cp in range(NCH // 2):
                nc.tensor.matmul(ps[:],
                                 lhsT=wqk_sb[:, 2 * cp:2 * cp + 2,
                                             c2 * P:(c2 + 1) * P],
                                 rhs=xcT[:, 2 * cp:2 * cp + 2,
                                         g * D:(g + 1) * D],
                                 start=(cp == 0), stop=(cp == NCH // 2 - 1),
                                 perf_mode=PM.DoubleRow)
            nc.scalar.activation(qkT[:, c2, g * D:(g + 1) * D], ps[:],
                                 AF.Identity, bias=bqk_sb[:, c2:c2 + 1])
        # threshold MLP hidden
        ps = psB.tile([P, D], F32, name="psB_t")
        for cp in range(NCH // 2):
            nc.tensor.matmul(ps[:],
                             lhsT=wt1_sb[:, 2 * cp:2 * cp + 2, :],
                             rhs=xcT[:, 2 * cp:2 * cp + 2, g * D:(g + 1) * D],
                             start=(cp == 0), stop=(cp == NCH // 2 - 1),
                             perf_mode=PM.DoubleRow)
        nc.scalar.activation(hT[:, g * D:(g + 1) * D], ps[:], AF.Gelu,
                             bias=bt1_sb[:], scale=1.0 / ST)
        for it in range(g * 4, g * 4 + 4):
            ps = psB.tile([P, D], F32, name="psB_t")
            nc.tensor.matmul(ps[:, :1], lhsT=hT[:, it * P:(it + 1) * P],
                             rhs=wt2_sb[:], start=True, stop=True)
            nc.scalar.activation(tcols[:, it:it + 1], ps[:, :1], AF.Identity,
                                 bias=bt2_sb[:])
            if has_u:
                psu = psB.tile([P, D], F32, name="psB_t")
                for ci in range(NCH):
                    nc.tensor.matmul(
                        psu[:, :1],
                        lhsT=qkT[:, ci, it * P:(it + 1) * P],
                        rhs=uv_sb[:, ci:ci + 1],
                        start=(ci == 0), stop=(ci == NCH - 1))
                # u = (Qk/SQ) . bk  (uv holds bk/1; psum = SQ*u)
                nc.vector.tensor_scalar(u_sb[:, it:it + 1], psu[:, :1],
                                        1.0 / SQ, None, op0=ALU.mult)

    # ==================== Phase 3: scores / topk / output ====================
    if has_u:
        tadj = stats.tile([P, NTQ], F32, name="tadj")
        nc.vector.tensor_tensor(tadj[:], tcols[:], u_sb[:], op=ALU.subtract)
        nc.scalar.activation(et[:], tadj[:], AF.Exp)
        nc.scalar.activation(eu[:], u_sb[:], AF.Exp)
    else:
        nc.scalar.activation(et[:], tcols[:], AF.Exp)

    sencs = (senc0, senc1)

    def scores_tile2(it):
        senc = sencs[it % 2]
        hi = senc[:].bitcast(FP8).rearrange("p (n four) -> p n four", four=4)
        for blk in range(4):
            col0 = blk * 2 * D
            ps = psA.tile([P, 2 * D], F32, name="psA_t")
            for half in range(2):
                ch0 = col0 + half * D
                for cp in range(NCH // 2):
                    nc.tensor.matmul(
                        ps[:, half * D:(half + 1) * D],
                        lhsT=qkT[:, 2 * cp:2 * cp + 2, it * P:(it + 1) * P],
                        rhs=ctxcT[:, 2 * cp:2 * cp + 2, ch0:ch0 + D],
                        start=(cp == 0), stop=(cp == NCH // 2 - 1),
                        perf_mode=PM.DoubleRow)
            nc.scalar.activation(hi[:, col0:col0 + 2 * D, 3], ps[:], AF.Exp,
                                 scale=1.0 / SQ)

        v8u = fpool.tile([P, 8], U32, name="v8u")
        nc.vector.max(v8u[:], senc[:])
        idx5 = fpool.tile([P, K5], U32, name="idx5")
        nc.vector.tensor_scalar(idx5[:], v8u[:, :K5], 7, 0xFFF,
                                op0=ALU.logical_shift_right,
                                op1=ALU.bitwise_and)
        # fp8 value byte lives in byte 3 of each packed u32 word
        vbf = v8u[:].bitcast(FP8).rearrange(
            "p (n four) -> p n four", four=4)[:, 0:K5, 3]
        kept = fpool.tile([P, K5], F32, name="kept")
        nc.vector.tensor_scalar(kept[:], vbf, et[:, it:it + 1], None,
                                op0=ALU.is_gt)
        w5 = fpool.tile([P, K5], F32, name="w5")
        sw = fpool.tile([P, 1], F32, name="sw")
        escale = eu[:, it:it + 1] if has_u else 1.0
        nc.vector.affine_mul_reduce(w5[:], sw[:], vbf, kept[:],
                                    scale=escale, bias=-1.0)
        t1 = fpool.tile([P, 1], F32, name="t1")
        nc.vector.tensor_scalar(t1[:], sw[:], SV, SV * M, op0=ALU.mult,
                                op1=ALU.add)
        rd = fpool.tile([P, 1], F32, name="rd")
        nc.vector.reciprocal(rd[:], t1[:])

        g_sb = gpool.tile([P, K5, D], FP8, name="g_sb")
        for j in range(K5):
            nc.gpsimd.indirect_dma_start(
                out=g_sb[:, j, :], out_offset=None, in_=vpd[:, :],
                in_offset=IndirectOffsetOnAxis(ap=idx5[:, j:j + 1], axis=0))

        dg5 = fpool.tile([P, K5, P], FP8, name="dg5")
        for j in range(K5):
            nc.vector.tensor_scalar(dg5[:, j, :], ident8[:], w5[:, j:j + 1],
                                    None, op0=ALU.mult)

        ps_a = psB.tile([P, D], F32, name="psB_t")
        nc.tensor.matmul(ps_a[:], lhsT=dg5[:, 0:2, :], rhs=g_sb[:, 0:2, :],
                         start=True, stop=False, perf_mode=PM.DoubleRow)
        nc.tensor.matmul(ps_a[:], lhsT=dg5[:, 2:4, :], rhs=g_sb[:, 2:4, :],
                         start=False, stop=False, perf_mode=PM.DoubleRow)
        nc.tensor.matmul(ps_a[:], lhsT=dg5[:, 4, :], rhs=g_sb[:, 4, :],
                         start=False, stop=False)
        if add_bias_out:
            dgd = fpool.tile([P, P], BF16, name="dgd")
            nc.vector.tensor_scalar(dgd[:], ident8[:], t1[:], None,
                                    op0=ALU.mult)
            nc.tensor.matmul(ps_a[:], lhsT=dgd[:], rhs=bob_sb[:],
                             start=False, stop=False)
        nc.tensor.matmul(ps_a[:], lhsT=ones_row[:], rhs=svp_row[:],
                         start=False, stop=True)

        o_sb = opool.tile([P, D], F32, name="o_sb")
        nc.scalar.activation(o_sb[:], ps_a[:], AF.Identity, scale=rd[:])
        nc.gpsimd.tensor_add(o_sb[:], o_sb[:], xraw[:, it, :])
        nc.sync.dma_start(out[it * P:(it + 1) * P, :], o_sb[:])

    for it in range(NTQ):
        scores_tile2(it)

    es.close()


_CACHE = {}


def get_compiled(flags):
    if flags in _CACHE:
        return _CACHE[flags]
    nc = bacc.Bacc("TRN2", target_bir_lowering=False, debug=False,
                   num_devices=8)
    with tile.TileContext(nc) as tc:
        build_core_program(tc, flags)
    nc.compile()
    _CACHE[flags] = nc
    return nc


def _fp8(a):
    return np.clip(np.asarray(a, np.float32), -240.0, 240.0).astype(
        ml_dtypes.float8_e4m3fn)


def make_in_maps(x, context, Wq, bq, Wk, bk, Wv, bv, Wt1, bt1, Wt2, bt2,
                 Wp, bp, g1, b1, g2, b2):
    f = np.float32
    x = np.asarray(x, f)
    context = np.asarray(context, f)
    Wq, bq, Wk, bk, Wv, bv = [np.asarray(a, f) for a in (Wq, bq, Wk, bk, Wv, bv)]
    Wt1, bt1, Wt2, bt2 = [np.asarray(a, f) for a in (Wt1, bt1, Wt2, bt2)]
    Wp, bp, g1, b1, g2, b2 = [np.asarray(a, f) for a in (Wp, bp, g1, b1, g2, b2)]

    scale = 1.0 / math.sqrt(D)
    bf = ml_dtypes.bfloat16

    Wqp = g1[:, None] * Wq
    bqp = b1 @ Wq + bq
    Wkp = g2[:, None] * Wk
    bkp = b2 @ Wk + bk

    WQK = (Wqp @ Wkp.T) * (scale * SQ)
    bqk = (Wkp @ bqp) * (scale * SQ)  # row-bias of scaled Qk
    u_vec = (Wqp @ bkp) * scale  # u = xn . u_vec + u_const (raw score units)
    u_const = float(bqp @ bkp) * scale
    has_u = bool(np.any(u_vec != 0) or u_const != 0)

    WVP = ((g2[:, None] * Wv) @ Wp) * SV
    bvp = (b2 @ Wv + bv) @ Wp
    Wt1p = (g1[:, None] * Wt1) * ST
    bt1p = b1 @ Wt1 + bt1
    bias_out = (bvp + bp).astype(f)
    add_bias_out = bool(np.any(bias_out != 0))

    in_maps = []
    for c in range(8):
        b, half = c // 2, c % 2
        in_maps.append({
            "xs": np.ascontiguousarray(x[b, half * NQ:(half + 1) * NQ]),
            "ctx": np.ascontiguousarray(context[b]),
            "wqk": _fp8(WQK), "wvp": _fp8(WVP),
            "wvpb": WVP.astype(bf),
            "wt1": _fp8(Wt1p),
            "wt2": Wt2.astype(bf).reshape(H, 1),
            "bqk": bqk.astype(f)[:, None],
            "bt1": bt1p.astype(f)[:, None],
            "bt2": np.full((P, 1), float(bt2.reshape(-1)[0]), f),
            "bo": bias_out[None, :].astype(bf),
            "uv": u_vec.astype(bf)[:, None],
        })
    # u_const is folded into bt2-like... it shifts u uniformly: adjust via
    # the et/eu path by adding to u; implemented by biasing uv matmul output.
    # Since u_const is zero for all realistic inputs (bq=0 or bk=0 gives 0),
    # assert to be safe.
    if has_u and u_const != 0:
        # fold constant into thresholds: v > t  <=>  s + u' + uc > t
        # exp(v) = e^s * e^u' * e^uc ; adjust via eu scale on host is not
        # possible (per-row); fall back: add uc to every u via bt-style bias.
        # We implement by adding uc into u_sb through uv matmul bias -- not
        # supported; instead shift tcols by -uc and scale weights by e^uc:
        raise NotImplementedError("nonzero bq@bk constant not supported")
    return in_maps, (add_bias_out, has_u)


def assemble(results):
    out = np.empty((4, 2 * NQ, D), np.float32)
    for c in range(8):
        b, half = c // 2, c % 2
        out[b, half * NQ:(half + 1) * NQ] = results[c]["out"]
    return out


def kernel(**inputs):
    from concourse.bass_utils import run_bass_kernel_spmd
    in_maps, flags = make_in_maps(**inputs)
    nc = get_compiled(flags)
    res = run_bass_kernel_spmd(nc, in_maps, core_ids=list(range(8)))
    return assemble(res.results)


# revision 26
# speedup vs baseline: 1.1757x; 1.1757x over previous
"""DynamicCrossAttention Trainium2 kernel (per-core builder + host wrapper).

Sharding: 8 shards = (B=4 batches) x (N=4096 query rows split in 2).
Each core: 2048 query rows of one batch, full context of that batch.

Key structure (v2):
  - K projection eliminated: S = xn @ WQK @ cn^T with WQK = (g1*Wq)(g2*Wk)^T/sqrt(C)
    folded on host (plus exact rank-1 bias corrections when biases nonzero).
  - fp8 (e4m3) DoubleRow matmuls for Qk / VP / MLP / scores / attn-out.
  - Top-k: scores drained by ACT as Exp(s) in bf16 into the HIGH u16 halves of a
    u32 tile whose LOW halves hold a column-index iota. One DVE Max8 over the
    u32 view returns (value|index) pairs: no MaxIndex scan needed.
  - Weights: w_j = e^{v_j}-1 (thresholded); denominator = M + sum w.
    out = (sum_j w_j VP[idx_j] + sum_r VP_r)/den + x.
"""

import math
import sys

sys.path.insert(0, "/opt/trn_rl_repo")

import numpy as np
import ml_dtypes

import concourse.bass as bass
import concourse.tile as tile
import concourse.mybir as mybir
from concourse import bacc
from concourse.bass import IndirectOffsetOnAxis
from concourse.masks import make_identity

F32 = mybir.dt.float32
BF16 = mybir.dt.bfloat16
FP8 = mybir.dt.float8e4
U32 = mybir.dt.uint32
U16 = mybir.dt.uint16
AF = mybir.ActivationFunctionType
ALU = mybir.AluOpType
PM = mybir.MatmulPerfMode

P = 128
D = 512
H = 128  # threshold MLP hidden
NQ = 2048  # query rows per core
M = 4096  # context rows
K5 = 5
EPS = 1e-5
NCH = D // P  # 4 feature chunks
NTQ = NQ // P  # 16 query tiles
NTC = M // P  # 32 context row tiles

SQ = 64.0  # scale folded into WQK (scores psum = SQ * s)
SV = 4.0  # scale folded into WVP (vp rows = SV * VP)
ST = 16.0  # scale folded into Wt1


def build_core_program(tc, flags):
    add_bias_out, has_u = flags
    nc = tc.nc

    xs = nc.dram_tensor("xs", [NQ, D], BF16, kind="ExternalInput").ap()
    ctx = nc.dram_tensor("ctx", [M, D], BF16, kind="ExternalInput").ap()
    wqk_d = nc.dram_tensor("wqk", [D, D], FP8, kind="ExternalInput").ap()
    wvp_d = nc.dram_tensor("wvp", [D, D], FP8, kind="ExternalInput").ap()
    wvpb_d = nc.dram_tensor("wvpb", [D, D], BF16, kind="ExternalInput").ap()
    wt1_d = nc.dram_tensor("wt1", [D, H], FP8, kind="ExternalInput").ap()
    wt2_d = nc.dram_tensor("wt2", [H, 1], BF16, kind="ExternalInput").ap()
    bqk_d = nc.dram_tensor("bqk", [D, 1], F32, kind="ExternalInput").ap()
    bt1_d = nc.dram_tensor("bt1", [H, 1], F32, kind="ExternalInput").ap()
    bt2_d = nc.dram_tensor("bt2", [P, 1], F32, kind="ExternalInput").ap()
    bo_d = nc.dram_tensor("bo", [1, D], BF16, kind="ExternalInput").ap()
    uv_d = nc.dram_tensor("uv", [D, 1], BF16, kind="ExternalInput").ap()
    out = nc.dram_tensor("out", [NQ, D], F32, kind="ExternalOutput").ap()
    vpd = nc.dram_tensor("vpd", [M, D], FP8, kind="Internal").ap()

    from contextlib import ExitStack
    es = ExitStack()
    const = es.enter_context(tc.tile_pool(name="const", bufs=1))
    wpool = es.enter_context(tc.tile_pool(name="wpool", bufs=1))
    big = es.enter_context(tc.tile_pool(name="big", bufs=1))
    ldpool = es.enter_context(tc.tile_pool(name="ld", bufs=3))
    npool = es.enter_context(tc.tile_pool(name="np", bufs=3))
    xnpool = es.enter_context(tc.tile_pool(name="xnp", bufs=4))
    stats = es.enter_context(tc.tile_pool(name="stats", bufs=8))
    vpool = es.enter_context(tc.tile_pool(name="vpool", bufs=3))
    fpool = es.enter_context(tc.tile_pool(name="fpool", bufs=4))
    gpool = es.enter_context(tc.tile_pool(name="gpool", bufs=3))
    opool = es.enter_context(tc.tile_pool(name="op", bufs=3))
    psA = es.enter_context(tc.tile_pool(name="psA", bufs=2, space="PSUM"))
    psB = es.enter_context(tc.tile_pool(name="psB", bufs=2, space="PSUM"))
    psT = es.enter_context(tc.tile_pool(name="psT", bufs=2, space="PSUM"))
    # PSUM budget: psS 4 banks + psB 2 + psT 2 = 8

    # ---- constants ----
    ident8 = const.tile([P, P], FP8, name="ident8")
    make_identity(nc, ident8[:])
    identb = const.tile([P, P], BF16, name="identb")
    make_identity(nc, identb[:])
    ones_row = const.tile([1, P], BF16, name="ones_row")
    nc.vector.memset(ones_row[:], 1.0)
    eps_c = const.tile([P, 1], F32, name="eps_c")
    nc.vector.memset(eps_c[:], EPS)

    # ---- weights to SBUF ----
    wqk_sb = wpool.tile([P, NCH, D], FP8, name="wqk_sb")
    nc.scalar.dma_start(wqk_sb[:], wqk_d.rearrange("(c p) d -> p c d", p=P))
    wvp_sb = wpool.tile([P, NCH, D], FP8, name="wvp_sb")
    nc.scalar.dma_start(wvp_sb[:], wvp_d.rearrange("(c p) d -> p c d", p=P))
    wvpb_sb = wpool.tile([P, NCH, D], BF16, name="wvpb_sb")
    nc.scalar.dma_start(wvpb_sb[:], wvpb_d.rearrange("(c p) d -> p c d", p=P))
    wt1_sb = wpool.tile([P, NCH, H], FP8, name="wt1_sb")
    nc.scalar.dma_start(wt1_sb[:], wt1_d.rearrange("(c p) d -> p c d", p=P))
    wt2_sb = wpool.tile([H, 1], BF16, name="wt2_sb")
    nc.scalar.dma_start(wt2_sb[:], wt2_d)
    bqk_sb = wpool.tile([P, NCH], F32, name="bqk_sb")
    nc.scalar.dma_start(bqk_sb[:], bqk_d.rearrange("(c p) o -> p (c o)", p=P))
    bt1_sb = wpool.tile([H, 1], F32, name="bt1_sb")
    nc.scalar.dma_start(bt1_sb[:], bt1_d)
    bt2_sb = wpool.tile([P, 1], F32, name="bt2_sb")
    nc.scalar.dma_start(bt2_sb[:], bt2_d)
    bo_sb = wpool.tile([1, D], BF16, name="bo_sb")
    nc.scalar.dma_start(bo_sb[:], bo_d)
    uv_sb = wpool.tile([P, NCH], BF16, name="uv_sb")
    nc.scalar.dma_start(uv_sb[:], uv_d.rearrange("(c p) o -> p (c o)", p=P))

    # ---- persistent tensors ----
    xraw = big.tile([P, NTQ, D], BF16, name="xraw")
    ctxcT = big.tile([P, NCH, M], FP8, name="ctxcT")
    qkT = big.tile([P, NCH, NQ], FP8, name="qkT")
    hT = big.tile([P, NQ], BF16, name="hT")
    tcols = big.tile([P, NTQ], F32, name="tcols")
    u_sb = big.tile([P, NTQ], F32, name="u_sb")
    et = big.tile([P, NTQ], F32, name="et")
    eu = big.tile([P, NTQ], F32, name="eu")
    sparts = big.tile([P, NCH, NTC // 4], F32, name="sparts")
    sctx = big.tile([P, NCH], F32, name="sctx")
    sctx_bf = big.tile([P, NCH], BF16, name="sctx_bf")
    svp_row = big.tile([1, D], BF16, name="svp_row")
    senc0 = big.tile([P, M], U32, name="senc0")
    senc1 = big.tile([P, M], U32, name="senc1")
    senc2 = big.tile([P, M], U32, name="senc2")

    # index iota (idx << 7) into the score-encode buffers; the Exp drain
    # overwrites byte 3 of each word with the fp8 value, bits 0..6 stay zero
    # so DVE Max8's internal mantissa rounding (step <= 128) is a no-op.
    for senc in (senc0, senc1, senc2):
        nc.gpsimd.iota(senc[:], pattern=[[128, M]], base=0,
                       channel_multiplier=0)

    if add_bias_out:
        psb = psB.tile([P, D], F32, name="psB_t")
        nc.tensor.matmul(psb[:], lhsT=ones_row[:], rhs=bo_sb[:], start=True,
                         stop=True)
        bob_sb = big.tile([P, D], BF16, name="bob_sb")
        nc.scalar.activation(bob_sb[:], psb[:], AF.Copy, bias=0.0)

    # ==================== Phase 1: layernorm stats+apply ====================
    def ln_group(raws, outs):
        mv4 = stats.tile([P, 4, 2], F32, name="mv4")
        for b in range(4):
            bn6 = stats.tile([P, 6], F32, name="bn6")
            nc.vector.bn_stats(bn6[:], raws[b])
            nc.vector.bn_aggr(mv4[:, b, :], bn6[:])
        sd4 = stats.tile([P, 4], F32, name="sd4")
        nc.scalar.activation(sd4[:], mv4[:, :, 1], AF.Sqrt, bias=eps_c[:])
        rstd4 = stats.tile([P, 4], F32, name="rstd4")
        nc.vector.reciprocal(rstd4[:], sd4[:])
        nmr4 = stats.tile([P, 4], F32, name="nmr4")
        nc.vector.tensor_mul(nmr4[:], mv4[:, :, 0], rstd4[:])
        nc.vector.tensor_scalar_mul(nmr4[:], nmr4[:], -1.0)
        for b in range(4):
            nc.vector.tensor_scalar(outs[b], raws[b], rstd4[:, b:b + 1],
                                    nmr4[:, b:b + 1], op0=ALU.mult, op1=ALU.add)

    # ==================== ctx pipeline (per 4-row-tile group) ====================
    # load -> layernorm -> transpose into ctxcT (+ column sums) -> VP -> vpd;
    # x loads and x layernorm groups are interleaved into the same stretch
    # (same Sqrt act table).
    xn4s = []

    def x_ln(g):
        xn4 = xnpool.tile([P, 4, D], BF16, name="xn4")
        ln_group([xraw[:, g * 4 + b, :] for b in range(4)],
                 [xn4[:, b, :] for b in range(4)])
        xn4s.append(xn4)

    for g in range(NTC // 4):
        raw4 = ldpool.tile([P, 4, D], BF16, name="raw4")
        nc.sync.dma_start(
            raw4[:], ctx[g * 4 * P:(g + 1) * 4 * P, :].rearrange(
                "(b p) d -> p b d", p=P))
        if g < 4:
            nc.scalar.dma_start(
                xraw[:, g * 4:(g + 1) * 4, :],
                xs[g * 4 * P:(g + 1) * 4 * P, :].rearrange(
                    "(b p) d -> p b d", p=P))
        cn4 = npool.tile([P, 4, D], BF16, name="cn4")
        ln_group([raw4[:, b, :] for b in range(4)],
                 [cn4[:, b, :] for b in range(4)])
        for c in range(NCH):
            pt = psT.tile([P, D], BF16, name="pt")
            for b in range(4):
                nc.tensor.transpose(pt[:, b * P:(b + 1) * P],
                                    cn4[:, b, c * P:(c + 1) * P],
                                    identb[:])
            nc.scalar.activation(ctxcT[:, c, g * D:(g + 1) * D], pt[:],
                                 AF.Copy, bias=0.0,
                                 accum_out=sparts[:, c, g:g + 1])
        vp_sb = vpool.tile([P, 4, D], FP8, name="vp_sb")
        for half in range(2):
            ps = psA.tile([P, 2 * D], F32, name="psA_t")
            for sub in range(2):
                jt = g * 4 + half * 2 + sub
                for cp in range(NCH // 2):
                    nc.tensor.matmul(ps[:, sub * D:(sub + 1) * D],
                                     lhsT=ctxcT[:, 2 * cp:2 * cp + 2,
                                                jt * P:(jt + 1) * P],
                                     rhs=wvp_sb[:, 2 * cp:2 * cp + 2, :],
                                     start=(cp == 0), stop=(cp == NCH // 2 - 1),
                                     perf_mode=PM.DoubleRow)
            nc.scalar.activation(vp_sb[:, half * 2:(half + 1) * 2, :], ps[:],
                                 AF.Copy, bias=0.0)
        nc.sync.dma_start(
            vpd[g * 4 * P:(g + 1) * 4 * P, :].rearrange(
                "(b p) d -> p b d", p=P),
            vp_sb[:])
        if 1 <= g <= 4:
            x_ln(g - 1)

    # svp_row = (column-sums of cn) @ WVP   (bf16 path for precision)
    for c in range(NCH):
        nc.vector.reduce_sum(sctx[:, c:c + 1], sparts[:, c, :],
                             axis=mybir.AxisListType.X)
    nc.vector.tensor_copy(sctx_bf[:], sctx[:])
    ps = psB.tile([P, D], F32, name="psB_t")
    for ci in range(NCH):
        nc.tensor.matmul(ps[:1, :], lhsT=sctx_bf[:, ci:ci + 1],
                         rhs=wvpb_sb[:, ci, :],
                         start=(ci == 0), stop=(ci == NCH - 1))
    nc.scalar.activation(svp_row[:], ps[:1, :], AF.Copy, bias=0.0)


    # x transposes -> xcT; Qk projection; threshold MLP (per 512-col group)
    xcT = big.tile([P, NCH, NQ], FP8, name="xcT")

    def x_group(g):
        xn4 = xn4s[g]
        for c in range(NCH):
            pt = psT.tile([P, D], BF16, name="pt")
            for b in range(4):
                nc.tensor.transpose(pt[:, b * P:(b + 1) * P],
                                    xn4[:, b, c * P:(c + 1) * P],
                                    identb[:])
            nc.vector.tensor_copy(xcT[:, c, g * D:(g + 1) * D], pt[:])
        for c2 in range(NCH):
            ps = psB.tile([P, D], F32, name="psB_t")
            for cp in range(NCH // 2):
                nc.tensor.matmul(ps[:],
                                 lhsT=wqk_sb[:, 2 * cp:2 * cp + 2,
                                             c2 * P:(c2 + 1) * P],
                                 rhs=xcT[:, 2 * cp:2 * cp + 2,
                                         g * D:(g + 1) * D],
                                 start=(cp == 0), stop=(cp == NCH // 2 - 1),
                                 perf_mode=PM.DoubleRow)
            nc.scalar.activation(qkT[:, c2, g * D:(g + 1) * D], ps[:],
                                 AF.Identity, bias=bqk_sb[:, c2:c2 + 1])
        ps = psB.tile([P, D], F32, name="psB_t")
        for cp in range(NCH // 2):
            nc.tensor.matmul(ps[:],
                             lhsT=wt1_sb[:, 2 * cp:2 * cp + 2, :],
                             rhs=xcT[:, 2 * cp:2 * cp + 2, g * D:(g + 1) * D],
                             start=(cp == 0), stop=(cp == NCH // 2 - 1),
                             perf_mode=PM.DoubleRow)
        nc.scalar.activation(hT[:, g * D:(g + 1) * D], ps[:], AF.Gelu,
                             bias=bt1_sb[:], scale=1.0 / ST)
        for it in range(g * 4, g * 4 + 4):
            ps = psB.tile([P, D], F32, name="psB_t")
            nc.tensor.matmul(ps[:, :1], lhsT=hT[:, it * P:(it + 1) * P],
                             rhs=wt2_sb[:], start=True, stop=True)
            nc.scalar.activation(tcols[:, it:it + 1], ps[:, :1], AF.Identity,
                                 bias=bt2_sb[:])
            if has_u:
                psu = psB.tile([P, D], F32, name="psB_t")
                for ci in range(NCH):
                    nc.tensor.matmul(
                        psu[:, :1],
                        lhsT=qkT[:, ci, it * P:(it + 1) * P],
                        rhs=uv_sb[:, ci:ci + 1],
                        start=(ci == 0), stop=(ci == NCH - 1))
                nc.vector.tensor_scalar(u_sb[:, it:it + 1], psu[:, :1],
                                        1.0 / SQ, None, op0=ALU.mult)
        gs = slice(g * 4, g * 4 + 4)
        if has_u:
            tadj = stats.tile([P, 4], F32, name="tadj")
            nc.vector.tensor_tensor(tadj[:], tcols[:, gs], u_sb[:, gs],
                                    op=ALU.subtract)
            nc.scalar.activation(et[:, gs], tadj[:], AF.Exp)
            nc.scalar.activation(eu[:, gs], u_sb[:, gs], AF.Exp)
        else:
            nc.scalar.activation(et[:, gs], tcols[:, gs], AF.Exp)

    # ==================== Phase 3: scores / topk / output ====================
    sencs = (senc0, senc1, senc2)

    def scores_front(it):
        senc = sencs[it % 3]
        hi = senc[:].bitcast(FP8).rearrange("p (n four) -> p n four", four=4)
        for blk in range(4):
            col0 = blk * 2 * D
            ps = psA.tile([P, 2 * D], F32, name="psA_t")
            for half in range(2):
                ch0 = col0 + half * D
                for cp in range(NCH // 2):
                    nc.tensor.matmul(
                        ps[:, half * D:(half + 1) * D],
                        lhsT=qkT[:, 2 * cp:2 * cp + 2, it * P:(it + 1) * P],
                        rhs=ctxcT[:, 2 * cp:2 * cp + 2, ch0:ch0 + D],
                        start=(cp == 0), stop=(cp == NCH // 2 - 1),
                        perf_mode=PM.DoubleRow)
            nc.scalar.activation(hi[:, col0:col0 + 2 * D, 3], ps[:], AF.Exp,
                                 scale=1.0 / SQ)

        v8u = fpool.tile([P, 8], U32, name="v8u")
        nc.vector.max(v8u[:], senc[:])
        idx5 = fpool.tile([P, K5], U32, name="idx5")
        nc.vector.tensor_scalar(idx5[:], v8u[:, :K5], 7, 0xFFF,
                                op0=ALU.logical_shift_right,
                                op1=ALU.bitwise_and)
        # fp8 value byte lives in byte 3 of each packed u32 word
        vbf = v8u[:].bitcast(FP8).rearrange(
            "p (n four) -> p n four", four=4)[:, 0:K5, 3]
        kept = fpool.tile([P, K5], F32, name="kept")
        nc.gpsimd.tensor_scalar(kept[:], vbf, et[:, it:it + 1], None,
                                op0=ALU.is_gt)
        w5 = fpool.tile([P, K5], F32, name="w5")
        sw = fpool.tile([P, 1], F32, name="sw")
        escale = eu[:, it:it + 1] if has_u else 1.0
        nc.vector.affine_mul_reduce(w5[:], sw[:], vbf, kept[:],
                                    scale=escale, bias=-1.0)
        # t1 = SV*(sw + M), rounded through bf16 so that the diag(t1) x-fold
        # and rd = 1/t1 use the exact same value (keeps the +x residual exact
        # up to x's own bf16 rounding)
        t1 = fpool.tile([P, 1], F32, name="t1")
        nc.vector.tensor_scalar(t1[:], sw[:], SV, SV * M, op0=ALU.mult,
                                op1=ALU.add)
        t1b = fpool.tile([P, 1], BF16, name="t1b")
        nc.vector.tensor_copy(t1b[:], t1[:])
        t1f = fpool.tile([P, 1], F32, name="t1f")
        nc.vector.tensor_copy(t1f[:], t1b[:])
        rd = fpool.tile([P, 1], F32, name="rd")
        nc.vector.reciprocal(rd[:], t1f[:])
        dgx = fpool.tile([P, P], BF16, name="dgx")
        nc.vector.tensor_scalar(dgx[:], identb[:], t1f[:], None, op0=ALU.mult)

        g_sb = gpool.tile([P, K5, D], FP8, name="g_sb")
        for j in range(K5):
            nc.gpsimd.indirect_dma_start(
                out=g_sb[:, j, :], out_offset=None, in_=vpd[:, :],
                in_offset=IndirectOffsetOnAxis(ap=idx5[:, j:j + 1], axis=0))
        return (it, w5, rd, dgx, g_sb)

    def scores_back(st):
        it, w5, rd, dgx, g_sb = st
        dg5 = fpool.tile([P, K5, P], FP8, name="dg5")
        for j in range(4):
            nc.vector.tensor_scalar(dg5[:, j, :], ident8[:], w5[:, j:j + 1],
                                    None, op0=ALU.mult)
        for j in range(4, K5):
            nc.scalar.activation(dg5[:, j, :], ident8[:], AF.Copy,
                                 scale=w5[:, j:j + 1])

        ps_a = psB.tile([P, D], F32, name="psB_t")
        nc.tensor.matmul(ps_a[:], lhsT=dg5[:, 0:2, :], rhs=g_sb[:, 0:2, :],
                         start=True, stop=False, perf_mode=PM.DoubleRow)
        nc.tensor.matmul(ps_a[:], lhsT=dg5[:, 2:4, :], rhs=g_sb[:, 2:4, :],
                         start=False, stop=False, perf_mode=PM.DoubleRow)
        nc.tensor.matmul(ps_a[:], lhsT=dg5[:, 4, :], rhs=g_sb[:, 4, :],
                         start=False, stop=False)
        if add_bias_out:
            nc.tensor.matmul(ps_a[:], lhsT=dgx[:], rhs=bob_sb[:],
                             start=False, stop=False)
        # residual: ps_a += t1 * x  (divides back out through the rd scale)
        nc.tensor.matmul(ps_a[:], lhsT=dgx[:], rhs=xraw[:, it, :],
                         start=False, stop=False)
        nc.tensor.matmul(ps_a[:], lhsT=ones_row[:], rhs=svp_row[:],
                         start=False, stop=True)

        if it % 2 == 0:
            ocell[0] = opool.tile([P, 2, D], F32, name="o_sb")
        o_sb = ocell[0]
        nc.scalar.activation(o_sb[:, it % 2, :], ps_a[:], AF.Identity,
                             scale=rd[:])
        if it % 2 == 1:
            nc.sync.dma_start(
                out[(it - 1) * P:(it + 1) * P, :].rearrange(
                    "(b p) d -> p b d", p=P),
                o_sb[:])

    # x projections fully before scores (keeps the ACT table on one set per
    # phase); scores are software-pipelined: back(it-1) after front(it).
    ocell = [None]
    for g in range(NTQ // 4):
        x_group(g)
    pend = None
    for it in range(NTQ):
        st = scores_front(it)
        if pend is not None:
            scores_back(pend)
        pend = st
    scores_back(pend)

    es.close()


_CACHE = {}


def get_compiled(flags):
    if flags in _CACHE:
        return _CACHE[flags]
    nc = bacc.Bacc("TRN2", target_bir_lowering=False, debug=False,
                   num_devices=8)
    with tile.TileContext(nc) as tc:
        build_core_program(tc, flags)
    nc.compile()
    _CACHE[flags] = nc
    return nc


def _fp8(a):
    return np.clip(np.asarray(a, np.float32), -240.0, 240.0).astype(
        ml_dtypes.float8_e4m3fn)


def make_in_maps(x, context, Wq, bq, Wk, bk, Wv, bv, Wt1, bt1, Wt2, bt2,
                 Wp, bp, g1, b1, g2, b2):
    f = np.float32
    x = np.asarray(x, f)
    context = np.asarray(context, f)
    Wq, bq, Wk, bk, Wv, bv = [np.asarray(a, f) for a in (Wq, bq, Wk, bk, Wv, bv)]
    Wt1, bt1, Wt2, bt2 = [np.asarray(a, f) for a in (Wt1, bt1, Wt2, bt2)]
    Wp, bp, g1, b1, g2, b2 = [np.asarray(a, f) for a in (Wp, bp, g1, b1, g2, b2)]

    scale = 1.0 / math.sqrt(D)
    bf = ml_dtypes.bfloat16

    Wqp = g1[:, None] * Wq
    bqp = b1 @ Wq + bq
    Wkp = g2[:, None] * Wk
    bkp = b2 @ Wk + bk

    WQK = (Wqp @ Wkp.T) * (scale * SQ)
    bqk = (Wkp @ bqp) * (scale * SQ)  # row-bias of scaled Qk
    u_vec = (Wqp @ bkp) * scale  # u = xn . u_vec + u_const (raw score units)
    u_const = float(bqp @ bkp) * scale
    has_u = bool(np.any(u_vec != 0) or u_const != 0)

    WVP = ((g2[:, None] * Wv) @ Wp) * SV
    bvp = (b2 @ Wv + bv) @ Wp
    Wt1p = (g1[:, None] * Wt1) * ST
    bt1p = b1 @ Wt1 + bt1
    bias_out = (bvp + bp).astype(f)
    add_bias_out = bool(np.any(bias_out != 0))

    in_maps = []
    for c in range(8):
        b, half = c // 2, c % 2
        in_maps.append({
            "xs": np.ascontiguousarray(x[b, half * NQ:(half + 1) * NQ]).astype(bf),
            "ctx": np.ascontiguousarray(context[b]).astype(bf),
            "wqk": _fp8(WQK), "wvp": _fp8(WVP),
            "wvpb": WVP.astype(bf),
            "wt1": _fp8(Wt1p),
            "wt2": Wt2.astype(bf).reshape(H, 1),
            "bqk": bqk.astype(f)[:, None],
            "bt1": bt1p.astype(f)[:, None],
            "bt2": np.full((P, 1), float(bt2.reshape(-1)[0]), f),
            "bo": bias_out[None, :].astype(bf),
            "uv": u_vec.astype(bf)[:, None],
        })
    # u_const is folded into bt2-like... it shifts u uniformly: adjust via
    # the et/eu path by adding to u; implemented by biasing uv matmul output.
    # Since u_const is zero for all realistic inputs (bq=0 or bk=0 gives 0),
    # assert to be safe.
    if has_u and u_const != 0:
        # fold constant into thresholds: v > t  <=>  s + u' + uc > t
        # exp(v) = e^s * e^u' * e^uc ; adjust via eu scale on host is not
        # possible (per-row); fall back: add uc to every u via bt-style bias.
        # We implement by adding uc into u_sb through uv matmul bias -- not
        # supported; instead shift tcols by -uc and scale weights by e^uc:
        raise NotImplementedError("nonzero bq@bk constant not supported")
    return in_maps, (add_bias_out, has_u)


def assemble(results):
    out = np.empty((4, 2 * NQ, D), np.float32)
    for c in range(8):
        b, half = c // 2, c % 2
        out[b, half * NQ:(half + 1) * NQ] = results[c]["out"]
    return out


def kernel(**inputs):
    from concourse.bass_utils import run_bass_kernel_spmd
    in_maps, flags = make_in_maps(**inputs)
    nc = get_compiled(flags)
    res = run_bass_kernel_spmd(nc, in_maps, core_ids=list(range(8)))
    return assemble(res.results)
